# revision 59
# baseline (speedup 1.0000x reference)
"""Multi-head self-attention block (B=4, N=2048, D=384, H=8, FF=1536) on 8 TRN2 cores.

Sharding: data-parallel over tokens. Core c handles batch b=c//2, query rows
[(c%2)*1024, (c%2+1)*1024). K/V are computed per-batch on each core (2x
replicated work, zero collectives). PE inputs are fp16 (f32 PSUM accumulation)
except the K/V projections, which run fp8(e4m3) in DoubleRowSwInterleave perf
mode (2 contraction rows per partition, 0.5 cycles/row; plain DoubleRow fails
TRN2 codegen -- s3_lw_dual_fp8 ldweights check) from host pair-interleaved
y/Wk/Wv. Weight operands additionally reverse columns within each 128-col
block (SwInterleave layout); y is shipped both ways since it is the kproj
ifmap (natural) and the vproj weights (reversed). The host pre-casts/pads
all inputs and unpads the f32 output.

Head padding: each 48-dim head occupies a 64-slot block:
  slots 0-47 = head dims, slot 48 = softmax-denominator slot, 49-63 = junk.
Q/K are feature-major [512pad, n] with that row layout (wq/wk host-padded with
zero rows). V is row-major "augmented": vaug[j] = [128 keys, 8*64] with
per-head block cols [V dims 0-47 | 1.0 | junk]; the ones column makes the
P@V matmul drop the softmax denominator into output col 48.

Attention datapath per head pair t (heads 2t, 2t+1):
  scores  S[j-tile, q] = K^T Q     (PSUM f32 [128,1024], keys on partitions)
  exp     head A tiles: ACT Exp; head B tiles: DVE Schraudolph bit-trick
          (out_i16 = s*A16 + B16, bitcast fp16, ~2%% rms exp error that
          largely cancels between softmax numerator and denominator) --
          splitting softmax exp across both engines is what keeps either
          from being the bottleneck; it is the largest single evacuation load.
  P@V     TRANSPOSED: out[q, v] = sum_j P[j,q] V[j,v] -- queries on output
          partitions (full 128-wide PE use; 49-wide moving dim). 8 i-tile
          accumulators per head packed at 64-col offsets into one PSUM bank.
          No memset: the first accumulator's j=0 matmul uses start=True,
          which zeroes its entire 2KB bank in hardware (verified: per-start
          bank zeroing clobbers neighbors, so ONLY i==0 starts); the other
          seven accumulate onto the zeroed bank with start=False.
  norm    denominator is per-partition (col 48): one batched DVE reciprocal
          per head ([128,8] over the stride-64 denominator columns), then
          ACT activation-Copy with per-partition scale (head A) / DVE
          tensor_scalar (head B) into o_r [128 q, 128] fp16.
  back    one PE transpose per (t, i) -> [128 v, 128 q] fp16 via bitcast
          views into a [128,1024] PSUM tile (4 transposes per tile, one per
          2KB bank so start-zeroing cannot clobber), then a single DVE
          scalar_tensor_tensor per 4 i-tiles adds the Q residual while
          copying to the padded feature-major ot_p.
ot_p is compacted 512->384 rows by 10 partition-moving SBUF->SBUF DMAs
(issued from the gpsimd sequencer), then the FFN (fp16 weights, f32 PSUM)
runs over compact dims: FFN2 accumulators m=0,1 are fed per-gelu; m=2
accumulates from retained hf tiles afterward.

Engine budget per core (cost model): PE ~122us (293k moving rows), ACT ~105us
(64 exp tiles + gelu + norms + q/k evacuations), DVE ~102us (64 Schraudolph
tiles + recips + residual adds + vproj evacuations), attention paced by the
3-slot scores-PSUM ring. PSUM: st 3x[128,1024] (6 banks) shared by scores /
projections / transposes / FFN1+m2, acc 2x[128,512] for P@V and FFN2.

Quirks: gpsimd ALU/memset ops touching PSUM fail neuronxcc codegen (gpsimd
does DMA issue + SBUF memsets only); DMA cannot read PSUM; plain DoubleRow
and nonzero tile_position columns are invalid ISA here; the Tile scheduler
reorders by readiness, so emission order is a hint. Input DMAs are spread
across the SP/ACT/gpsimd sequencers so the critical kproj(0,*)/qproj(0)
inputs land in ~2.5us instead of serializing on one queue.
"""

import math
import os
import numpy as np

B, N, D, H, DH, DFF = 4, 2048, 384, 8, 48, 1536
PH = 64            # padded per-head block
DP = H * PH        # 512 padded model dim
ROWS = 1024        # query rows per core
KD = D // 128      # 3 k-tiles over model dim (fp16 path)
KD8 = D // 128     # 3 k-tiles over model dim (fp8 DoubleRow path);
                   # DoubleRow packs 2 rows/partition, so tiles are 64-partition
TQ = DP // 128     # 4 tiles over padded dim (= head pairs)
NJ = N // 128      # 16 key tiles
NI = ROWS // 128   # 8 query i-tiles
NF = DFF // 128    # 12 ffn tiles
KH = DH + 1        # 49 cols per head block incl denominator col
SCALE = 1.0 / math.sqrt(D)

# Schraudolph fp16 exp: bitcast_f16(int16(s*A16 + B16)) ~= exp(s*SCALE)
A16 = SCALE * 1024.0 / math.log(2.0)
B16 = 15.0 * 1024.0 - 60.0

FP8_KV = os.environ.get("KERN_FP8KV", "1") == "1"
NOMEMSET = os.environ.get("KERN_NOMEMSET", "2")


# DMA segments to compact padded ot_p [512 rows] -> otc [384 rows]:
# (src_tile, src_row, dst_tile, dst_row, nrows)
def _compact_segs():
    segs = []
    for h in range(H):
        s_lo, d, left, off = 64 * (h % 2), DH * h, DH, 0
        while left:
            n = min(left, 128 - ((d + off) % 128))
            segs.append((h // 2, s_lo + off, (d + off) // 128, (d + off) % 128, n))
            off += n
            left -= n
    return segs


CSEGS = _compact_segs()

_CACHE = {}


def _build():
    from contextlib import ExitStack
    import concourse.bass as bass
    import concourse.bacc as bacc
    import concourse.tile as tile
    import concourse.mybir as mybir

    F32 = mybir.dt.float32
    F16 = mybir.dt.float16
    I16 = mybir.dt.int16
    F8 = mybir.dt.float8e4
    AF = mybir.ActivationFunctionType
    ALU = mybir.AluOpType
    DR = mybir.MatmulPerfMode.DoubleRowSwInterleave
    ts = bass.ts

    nc = bacc.Bacc(trn_type="TRN2", target_bir_lowering=False, debug=False)

    def din(name, shape, dt=F16):
        return nc.dram_tensor(name, shape, dt, kind="ExternalInput").ap()

    xT = din("xT", [D, ROWS])
    wqT = din("wqT", [D, DP])
    w1T = din("w1T", [D, DFF])
    w2T = din("w2T", [DFF, D])
    idT = din("idT", [128, 128])
    if FP8_KV:
        # pair-interleaved fp8 (row 64k+p holds dims {128k+2p, 128k+2p+1}).
        # "r" variants additionally reverse columns within each 128-col block
        # (DoubleRowSwInterleave weight layout). y is loaded both ways: as
        # kproj ifmap (natural) and as vproj weights (reversed).
        y8T = din("y8T", [192, 2 * N], F8)
        y8rT = din("y8rT", [192, 2 * N], F8)
        wk8T = din("wk8T", [192, 2 * DP], F8)
        wv8T = din("wv8T", [192, 2 * D], F8)
    else:
        yT = din("yT", [D, N])
        wkT = din("wkT", [D, DP])
        wvT = din("wvT", [D, D])
    o = nc.dram_tensor("o", [D, ROWS], F32, kind="ExternalOutput").ap()

    with tile.TileContext(nc) as tc, ExitStack() as ctx:
        sb = ctx.enter_context(tc.tile_pool(name="sb", bufs=1))
        ps = ctx.enter_context(tc.tile_pool(name="ps", bufs=1, space="PSUM"))

        # ---- persistent SBUF tiles ----
        xt = [sb.tile([128, ROWS], F16, tag="xt", bufs=3, name=f"xt{k}") for k in range(KD)]
        wq = [sb.tile([128, DP], F16, tag="wq", bufs=3, name=f"wq{k}") for k in range(KD)]
        if FP8_KV:
            yt8 = [sb.tile([64, 2 * N], F8, tag="yt", bufs=3, name=f"yt8_{k}")
                   for k in range(KD8)]
            yt8r = [sb.tile([64, 2 * N], F8, tag="ytr", bufs=3, name=f"yt8r_{k}")
                    for k in range(KD8)]
            wk8 = [sb.tile([64, 2 * DP], F8, tag="wk", bufs=3, name=f"wk8_{k}")
                   for k in range(KD8)]
            wv8 = [sb.tile([64, 2 * D], F8, tag="wv", bufs=3, name=f"wv8_{k}")
                   for k in range(KD8)]
        else:
            yt = [sb.tile([128, N], F16, tag="yt", bufs=3, name=f"yt{k}") for k in range(KD)]
            wk = [sb.tile([128, DP], F16, tag="wk", bufs=3, name=f"wk{k}") for k in range(KD)]
            wv = [sb.tile([128, D], F16, tag="wv", bufs=3, name=f"wv{k}") for k in range(KD)]
        qt = [sb.tile([128, ROWS], F16, tag="qt", bufs=4, name=f"qt{t}") for t in range(TQ)]
        kt = [sb.tile([128, N], F16, tag="kt", bufs=4, name=f"kt{t}") for t in range(TQ)]
        vaug = [sb.tile([128, DP], F16, tag="va", bufs=16, name=f"va{j}") for j in range(NJ)]
        ident = sb.tile([128, 128], F16, tag="id", bufs=1, name="ident")
        ot_p = [sb.tile([128, ROWS], F16, tag="otp", bufs=4, name=f"otp{t}") for t in range(TQ)]
        otc = [sb.tile([128, ROWS], F16, tag="otc", bufs=3, name=f"otc{m}") for m in range(KD)]
        w1 = [sb.tile([128, DFF], F16, tag="w1", bufs=3, name=f"w1_{k}") for k in range(KD)]
        w2 = [sb.tile([128, D], F16, tag="w2", bufs=12, name=f"w2_{f}") for f in range(NF)]

        # ---- input loads, spread across sequencers (SP / ACT / gpsimd) ----
        if FP8_KV:
            for k in range(KD8):
                nc.sync.dma_start(out=wk8[k][:], in_=wk8T[ts(k, 64), :])
                nc.sync.dma_start(out=yt8[k][:, 0:1024], in_=y8T[ts(k, 64), 0:1024])
            for k in range(KD):
                nc.scalar.dma_start(out=wq[k][:], in_=wqT[ts(k, 128), :])
                nc.scalar.dma_start(out=xt[k][:], in_=xT[ts(k, 128), :])
            for k in range(KD8):
                nc.gpsimd.dma_start(out=yt8[k][:, 1024:2048], in_=y8T[ts(k, 64), 1024:2048])
            for k in range(KD8):
                nc.gpsimd.dma_start(out=yt8[k][:, 2048:4096], in_=y8T[ts(k, 64), 2048:4096])
            for k in range(KD8):
                nc.gpsimd.dma_start(out=yt8r[k][:, 0:1024], in_=y8rT[ts(k, 64), 0:1024])
                nc.gpsimd.dma_start(out=wv8[k][:], in_=wv8T[ts(k, 64), :])
            nc.sync.dma_start(out=ident[:], in_=idT[:, :])
            for k in range(KD8):
                nc.gpsimd.dma_start(out=yt8r[k][:, 1024:4096], in_=y8rT[ts(k, 64), 1024:4096])
        else:
            for k in range(KD):
                nc.sync.dma_start(out=wk[k][:], in_=wkT[ts(k, 128), :])
                nc.sync.dma_start(out=yt[k][:, 0:512], in_=yT[ts(k, 128), 0:512])
            for k in range(KD):
                nc.scalar.dma_start(out=wq[k][:], in_=wqT[ts(k, 128), :])
                nc.scalar.dma_start(out=xt[k][:], in_=xT[ts(k, 128), :])
            for k in range(KD):
                nc.gpsimd.dma_start(out=yt[k][:, 512:1024], in_=yT[ts(k, 128), 512:1024])
            for k in range(KD):
                nc.gpsimd.dma_start(out=wv[k][:], in_=wvT[ts(k, 128), :])
            nc.sync.dma_start(out=ident[:], in_=idT[:, :])
            for k in range(KD):
                nc.gpsimd.dma_start(out=yt[k][:, 1024:2048], in_=yT[ts(k, 128), 1024:2048])

        def load_ffn_weights():
            for f in range(NF):
                nc.sync.dma_start(out=w2[f][:], in_=w2T[ts(f, 128), :])
            for k in range(KD):
                nc.sync.dma_start(out=w1[k][:], in_=w1T[ts(k, 128), :])

        # ---- projections ([128,1024] st-tag PSUM tiles, 2 bank-groups) ----
        def qproj(t):
            p = ps.tile([128, 1024], F32, tag="st", bufs=3, name=f"psq{t}")
            for c in range(2):
                for k in range(KD):
                    nc.tensor.matmul(p[:, ts(c, 512)], wq[k][:, ts(t, 128)],
                                     xt[k][:, ts(c, 512)],
                                     start=(k == 0), stop=(k == KD - 1))
            nc.scalar.copy(qt[t][:], p[:])

        def kproj(t, half):
            p = ps.tile([128, 1024], F32, tag="st", bufs=3, name=f"psk{t}_{half}")
            for c in range(2):
                n = 2 * half + c
                if FP8_KV:
                    for k in range(KD8):
                        nc.tensor.matmul(
                            p[:, ts(c, 512)],
                            wk8[k][:, 256 * t:256 * (t + 1)],
                            yt8[k][:, 1024 * n:1024 * (n + 1)]
                                .rearrange("p (n s) -> p s n", s=2),
                            start=(k == 0), stop=(k == KD8 - 1), perf_mode=DR)
                else:
                    for k in range(KD):
                        nc.tensor.matmul(p[:, ts(c, 512)], wk[k][:, ts(t, 128)],
                                         yt[k][:, ts(n, 512)],
                                         start=(k == 0), stop=(k == KD - 1))
            nc.scalar.copy(kt[t][:, ts(half, 1024)], p[:])

        def vproj(j):
            p = ps.tile([128, 1024], F32, tag="st", bufs=3, name=f"psv{j}")
            if FP8_KV:
                for k in range(KD8):
                    nc.tensor.matmul(
                        p[:, 0:D],
                        yt8r[k][:, 256 * j:256 * (j + 1)],
                        wv8[k][:].rearrange("p (n s) -> p s n", s=2),
                        start=(k == 0), stop=(k == KD8 - 1), perf_mode=DR)
            else:
                for k in range(KD):
                    nc.tensor.matmul(p[:, 0:D], yt[k][:, ts(j, 128)], wv[k][:],
                                     start=(k == 0), stop=(k == KD - 1))
            va3 = vaug[j][:].rearrange("p (h e) -> p h e", h=H)
            ps3 = p[:, 0:D].rearrange("p (h e) -> p h e", h=H)
            if j % 2 == 0:
                nc.scalar.copy(va3[:, :, 0:DH], ps3[:, :, 0:DH])
            else:
                nc.vector.tensor_copy(va3[:, :, 0:DH], ps3[:, :, 0:DH])
            nc.gpsimd.memset(va3[:, :, DH:DH + 1], 1.0)

        kproj(0, 0)
        qproj(0)
        kproj(0, 1)
        for j in range(4):
            vproj(j)
        qproj(1)
        kproj(1, 0)
        kproj(1, 1)

        # background work emitted inside the attention j-loops: (t, j) -> fns
        bg = {}
        bg[(0, 1)] = [lambda: vproj(4), lambda: vproj(5)]
        bg[(0, 3)] = [lambda: vproj(6), lambda: vproj(7)]
        bg[(0, 5)] = [lambda: vproj(8), lambda: vproj(9)]
        bg[(0, 7)] = [lambda: vproj(10), lambda: vproj(11)]
        bg[(0, 9)] = [lambda: vproj(12), lambda: vproj(13)]
        bg[(0, 11)] = [lambda: vproj(14), lambda: vproj(15)]
        bg[(0, 13)] = [lambda: qproj(2)]
        bg[(1, 1)] = [lambda: kproj(2, 0)]
        bg[(1, 5)] = [lambda: kproj(2, 1)]
        bg[(1, 9)] = [lambda: qproj(3)]
        bg[(1, 13)] = [lambda: kproj(3, 0)]
        bg[(2, 1)] = [lambda: kproj(3, 1)]
        bg[(2, 5)] = [load_ffn_weights]

        # ---- attention ----
        # head-B exp tiles stolen by ACT in windows where it has slack
        ACT_STEAL = {(2, 5), (2, 11), (3, 5), (3, 11)}
        o_r = {}
        pending = None

        def drain_norm():
            t, accA, accB = pending
            rcs = {}
            for a, acc in ((0, accA), (1, accB)):
                # batched reciprocal of the 8 denominator columns (stride 64)
                rc = sb.tile([128, NI], F32, tag="rc", bufs=4, name=f"rc{t}_{a}")
                dens = acc[:].rearrange("p (i e) -> p i e", e=PH)[:, :, DH:DH + 1]
                nc.vector.reciprocal(rc[:], dens)
                rcs[a] = rc
            for i in range(NI):
                o_r[(t, i)] = sb.tile([128, 128], F16, tag="or", bufs=10,
                                      name=f"or{t}_{i}")
                nc.scalar.activation(
                    o_r[(t, i)][:, 0:KH],
                    accA[:, PH * i:PH * i + KH], AF.Copy,
                    scale=rcs[0][:, i:i + 1])
                nc.vector.tensor_scalar(
                    o_r[(t, i)][:, PH:PH + KH],
                    accB[:, PH * i:PH * i + KH],
                    rcs[1][:, i:i + 1], None, ALU.mult)

        def drain_transpose():
            t = pending[0]
            for q in range(NI // 4):
                # 4 transposes per PSUM tile (one per 2KB bank: no zero-region
                # clobber), then a single fused residual-add evacuates all 4
                tp = ps.tile([128, 1024], F32, tag="st", bufs=3, name=f"tp{t}_{q}")
                for u in range(4):
                    tpv = tp[:, 256 * u:256 * u + 64].bitcast(F16)
                    nc.tensor.transpose(tpv, o_r[(t, 4 * q + u)][:], ident[:])
                tp4 = tp[:].bitcast(F16).rearrange(
                    "p (b r) -> p b r", b=4)[:, :, 0:128]
                nc.vector.scalar_tensor_tensor(
                    ot_p[t][:, ts(q, 512)], tp4, 1.0, qt[t][:, ts(q, 512)],
                    ALU.mult, ALU.add)

        def drain_csegs():
            # column-halved so FFN1's c=0 phase (which reads otc[:, 0:512])
            # unblocks after only the first half of the final drain
            t = pending[0]
            for half in range(2):
                cl, ch = 512 * half, 512 * (half + 1)
                for st_, sr, dt_, dr, nr in CSEGS:
                    if st_ == t:
                        nc.gpsimd.dma_start(out=otc[dt_][dr:dr + nr, cl:ch],
                                            in_=ot_p[t][sr:sr + nr, cl:ch])

        def drain_pending():
            drain_norm()
            drain_transpose()
            drain_csegs()

        for t in range(TQ):
            accA = ps.tile([128, 512], F32, tag="acc", bufs=2, name=f"accA{t}")
            accB = ps.tile([128, 512], F32, tag="acc", bufs=2, name=f"accB{t}")
            if NOMEMSET == "0":
                nc.vector.memset(accA[:], 0.0)
                nc.vector.memset(accB[:], 0.0)
            for j in range(NJ):
                pe2 = []
                for a in range(2):
                    pe = sb.tile([128, 1024], F16, tag="pt", bufs=6,
                                 name=f"pe{t}_{j}_{a}")
                    stx = ps.tile([128, 1024], F32, tag="st", bufs=3,
                                  name=f"st{t}_{j}_{a}")
                    for c in range(2):
                        nc.tensor.matmul(
                            stx[:, ts(c, 512)],
                            kt[t][PH * a:PH * a + DH, ts(j, 128)],
                            qt[t][PH * a:PH * a + DH, ts(c, 512)],
                            start=True, stop=True)
                    if a == 1 and (t, j) not in ACT_STEAL:
                        nc.vector.tensor_scalar(
                            pe[:].bitcast(I16), stx[:], A16, B16,
                            ALU.mult, ALU.add)
                    else:
                        nc.scalar.activation(pe[:], stx[:], AF.Exp, scale=SCALE)
                    pe2.append(pe)
                for fn in bg.get((t, j), ()):
                    fn()
                if j == 2 and pending is not None:
                    drain_pending()
                for a, acc in ((0, accA), (1, accB)):
                    for i in range(NI):
                        nc.tensor.matmul(
                            acc[:, PH * i:PH * i + KH],
                            pe2[a][:, ts(i, 128)],
                            vaug[j][:, PH * (2 * t + a):PH * (2 * t + a) + KH],
                            start=(j == 0 and (NOMEMSET == "1" or (NOMEMSET == "2" and i == 0))),
                            stop=(j == NJ - 1),
                            skip_group_check=True)
            pending = (t, accA, accB)
        drain_pending()

        # ---- FFN (feature-major, compact): otc -> gelu(W1@otc) -> W2@hid + otc
        for c in range(2):
            # FFN2 accumulators m=0,1 are fed as soon as each gelu tile lands;
            # m=2 accumulates after the g-loop from retained hf tiles, so only
            # its 12 matmuls + add trail the final gelu.
            po = [ps.tile([128, 512], F32, tag="acc", bufs=2, name=f"po{c}_{m}")
                  for m in range(2)]
            hf = []
            for g in range(NF // 2):
                sg = ps.tile([128, 1024], F32, tag="st", bufs=3, name=f"sg{c}_{g}")
                for fi in range(2):
                    for k in range(KD):
                        nc.tensor.matmul(
                            sg[:, ts(fi, 512)],
                            w1[k][:, ts(g * 2 + fi, 128)], otc[k][:, ts(c, 512)],
                            start=(k == 0), stop=(k == KD - 1))
                h = sb.tile([128, 1024], F16, tag="hid", bufs=8, name=f"hf{c}_{g}")
                nc.scalar.activation(h[:], sg[:], AF.Gelu)
                hf.append(h)
                for m in range(2):
                    for fi in range(2):
                        nc.tensor.matmul(
                            po[m][:], w2[g * 2 + fi][:, ts(m, 128)],
                            h[:, ts(fi, 512)],
                            start=(g == 0 and fi == 0),
                            stop=(g == NF // 2 - 1 and fi == 1))
            po2 = ps.tile([128, 1024], F32, tag="st", bufs=3, name=f"po2_{c}")
            for g in range(NF // 2):
                for fi in range(2):
                    nc.tensor.matmul(
                        po2[:, 0:512], w2[g * 2 + fi][:, 256:384],
                        hf[g][:, ts(fi, 512)],
                        start=(g == 0 and fi == 0),
                        stop=(g == NF // 2 - 1 and fi == 1))
            for m in range(KD):
                src_ = po[m][:] if m < 2 else po2[:, 0:512]
                osb = sb.tile([128, 512], F32, tag="osb", bufs=3, name=f"osb{c}_{m}")
                nc.vector.tensor_add(osb[:], src_, otc[m][:, ts(c, 512)])
                nc.sync.dma_start(out=o[ts(m, 128), ts(c, 512)], in_=osb[:])

    nc.compile()
    return nc


def _interleave_rows(w):
    # [384, X] -> [192, 2X]: out[64k+p, 2n+s] = w[128k+2p+s, n]
    x = w.reshape(3, 64, 2, -1)                 # [k, p, s, n]
    x = np.transpose(x, (0, 1, 3, 2))           # [k, p, n, s]
    return np.ascontiguousarray(x.reshape(192, -1))


def _interleave_rows_rev(w):
    # as _interleave_rows, but columns reversed within each 128-col block
    # (DoubleRowSwInterleave weight layout)
    wr = np.ascontiguousarray(
        w.reshape(w.shape[0], -1, 128)[:, :, ::-1]).reshape(w.shape[0], -1)
    return _interleave_rows(wr)


def _prep_weights(Wq, Wk, Wv, W1, W2):
    def pad_rows(w):  # [384, X] -> [512, X]; head h dims at rows 64h..64h+47
        out = np.zeros((DP,) + w.shape[1:], dtype=w.dtype)
        out.reshape(H, PH, -1)[:, 0:DH] = w.reshape(H, DH, -1)
        return out

    f16 = np.float16
    wqT = np.ascontiguousarray(pad_rows(Wq).T).astype(f16)    # [384, 512]
    w1T = np.ascontiguousarray(W1.T).astype(f16)              # [384, 1536]
    w2T = np.ascontiguousarray(W2.T).astype(f16)              # [1536, 384]
    if FP8_KV:
        import ml_dtypes
        f8 = ml_dtypes.float8_e4m3fn
        wk8T = _interleave_rows_rev(np.ascontiguousarray(pad_rows(Wk).T)).astype(f8)
        wv8T = _interleave_rows(np.ascontiguousarray(Wv.T)).astype(f8)
        return wqT, wk8T, wv8T, w1T, w2T
    wkT = np.ascontiguousarray(pad_rows(Wk).T).astype(f16)    # [384, 512]
    wvT = np.ascontiguousarray(Wv.T).astype(f16)              # [384, 384]
    return wqT, wkT, wvT, w1T, w2T


def _run(in_maps, trace=False):
    from concourse.bass_utils import run_bass_kernel_spmd

    if "nc" not in _CACHE:
        _CACHE["nc"] = _build()
    try:
        return run_bass_kernel_spmd(_CACHE["nc"], in_maps, list(range(8)), trace=trace)
    except Exception:
        # one retry: absorbs transient device wedges (NRT_EXEC_UNIT_* from a
        # previous interrupted run on the shared tunneled devices). Once PJRT
        # marks a device unrecoverable the client is poisoned, so drop the
        # cached backends to force a fresh client before retrying.
        import time as _time
        last = None
        for delay in (10.0, 30.0):
            try:
                import jax
                import jax._src.xla_bridge as _xb
                jax.clear_caches()
                with _xb._backend_lock:
                    _xb._backends.clear()
                    _xb._backend_errors.clear()
            except Exception:
                pass
            _time.sleep(delay)
            try:
                return run_bass_kernel_spmd(_CACHE["nc"], in_maps,
                                            list(range(8)), trace=trace)
            except Exception as e:  # noqa
                last = e
        raise last


def _make_in_maps(x, y, Wq, Wk, Wv, W1, W2):
    x = np.asarray(x, dtype=np.float32)
    y = np.asarray(y, dtype=np.float32)
    wqT, wkX, wvX, w1T, w2T = _prep_weights(
        np.asarray(Wq, np.float32), np.asarray(Wk, np.float32),
        np.asarray(Wv, np.float32), np.asarray(W1, np.float32),
        np.asarray(W2, np.float32))
    ident = np.eye(128, dtype=np.float16)
    in_maps = []
    for c in range(8):
        b, half = c // 2, c % 2
        xs = x[b, half * ROWS:(half + 1) * ROWS]  # [1024, 384]
        m = {
            "xT": np.ascontiguousarray(xs.T).astype(np.float16),
            "wqT": wqT, "w1T": w1T, "w2T": w2T, "idT": ident,
        }
        yTc = np.ascontiguousarray(y[b].T)
        if FP8_KV:
            import ml_dtypes
            f8d = ml_dtypes.float8_e4m3fn
            y8 = yTc.astype(f8d)
            m["y8T"] = _interleave_rows(y8)
            m["y8rT"] = _interleave_rows_rev(y8)
            m["wk8T"] = wkX
            m["wv8T"] = wvX
        else:
            m["yT"] = yTc.astype(np.float16)
            m["wkT"] = wkX
            m["wvT"] = wvX
        in_maps.append(m)
    return in_maps


def _unshard(results):
    out = np.empty((B, N, D), np.float32)
    for c in range(8):
        oc = results[c]["o"]  # [384, 1024] compact feature-major
        out[c // 2, (c % 2) * ROWS:(c % 2 + 1) * ROWS, :] = oc.T
    return out


def kernel(x, y, Wq, Wk, Wv, W1, W2):
    res = _run(_make_in_maps(x, y, Wq, Wk, Wv, W1, W2))
    return _unshard(res.results)


def profile(x, y, Wq, Wk, Wv, W1, W2):
    """Run with NTFF tracing; returns exec_time_ns (or None)."""
    import concourse.bass_utils as bu
    orig = bu.upload_artifacts
    bu.upload_artifacts = lambda tmpdir: f"file://{tmpdir}"
    try:
        res = _run(_make_in_maps(x, y, Wq, Wk, Wv, W1, W2), trace=True)
    finally:
        bu.upload_artifacts = orig
    return res.exec_time_ns
